# revision 6
# baseline (speedup 1.0000x reference)
"""Trainium2 Bass kernel for nn_DepthSeparableConv2d_conv2_5.

Computation (per sample):
  y = relu(BN1(depthwise3x3(x) + dw_b));  y = prune(y, 4.0)   [per-(b,c) absmax]
  z = relu(BN2(pw_w @ y + pw_b));         z = prune(z, 0.001) [per-(b,o) absmax]

Mapping (8 NeuronCores, data-parallel over batch, 8 samples/core):
  - depthwise conv = 9 accumulating fp32r matmuls with diagonal [128,128]
    weight matrices (one per tap); the shift is an access-pattern offset on
    the rhs; BN1 scale is folded into the diagonal weights.
  - DW epilogue (bias+ReLU) on ScalarE reading 2 PSUM banks per op.
  - exact per-plane maxes via DVE tensor_scalar with accum_out (op1=max).
  - DW prune mask is folded into the pointwise lhsT (zero pruned rows).
  - pointwise 1x1 conv = fp32r GEMM, BN2 scale folded into pw weights,
    bias+ReLU on ScalarE, prune mask applied by GpSimd before the store.
"""

import numpy as np

import concourse.bass as bass
import concourse.mybir as mybir
import concourse.tile as tile
from concourse import bacc
from concourse.bass_utils import run_bass_kernel_spmd
from concourse.masks import make_identity

f32 = mybir.dt.float32
f32r = mybir.dt.float32r
Alu = mybir.AluOpType
Act = mybir.ActivationFunctionType
AxL = mybir.AxisListType

N_CORES = 8
B = 64
BPC = B // N_CORES  # samples per core
CIN, COUT = 128, 256
H = W = 56
HW = H * W  # 3136
WP = W + 2   # host-side zero-padded row width (fp32r needs even full-width taps)
HWP = H * WP # 3648
NT = 7      # pixel tiles per plane, 8 rows (448 px) each
TR = 8      # rows per pixel tile
EPS = 1e-5
DW_T, PW_T = 4.0, 0.001

# tap (0,0) first: it covers the full region, so it can carry start=True
TAPS = [(0, 0)] + [
    (dh, dw) for dh in (-1, 0, 1) for dw in (-1, 0, 1) if (dh, dw) != (0, 0)
]


def _affine(nc, pool, name, var, gamma, beta, mean, bvec):
    """s = gamma/sqrt(var+eps); t = (bvec - mean)*s + beta. All [128,1]."""
    tmp = pool.tile([128, 1], f32, tag=f"{name}_tmp")
    nc.vector.tensor_scalar(tmp[:], var, EPS, None, Alu.add)
    sq = pool.tile([128, 1], f32, tag=f"{name}_sq")
    nc.scalar.sqrt(sq[:], tmp[:])
    rc = pool.tile([128, 1], f32, tag=f"{name}_rc")
    nc.vector.reciprocal(rc[:], sq[:])
    s = pool.tile([128, 1], f32, tag=f"{name}_s")
    nc.vector.tensor_mul(s[:], gamma, rc[:])
    u = pool.tile([128, 1], f32, tag=f"{name}_u")
    nc.vector.tensor_sub(u[:], bvec, mean)
    nc.vector.tensor_mul(u[:], u[:], s[:])
    t = pool.tile([128, 1], f32, tag=f"{name}_t")
    nc.vector.tensor_add(t[:], u[:], beta)
    return s, t


def build():
    nc = bacc.Bacc(trn_type="TRN2", target_bir_lowering=False, debug=False)

    # x is consumed only by fp32r matmuls; declaring it float32r end-to-end
    # satisfies the verifier's "rounded to FP32r" producer rule with a plain
    # HWDGE same-dtype DMA (bits are ordinary fp32; the PE rounds on read).
    x_d = nc.dram_tensor("x", [BPC, CIN, HWP], f32r, kind="ExternalInput").ap()
    dww_d = nc.dram_tensor("dw_w", [CIN, 9], f32, kind="ExternalInput").ap()
    dwb_d = nc.dram_tensor("dw_b", [CIN, 1], f32, kind="ExternalInput").ap()
    bn1 = {
        k: nc.dram_tensor(f"bn1_{k}", [CIN, 1], f32, kind="ExternalInput").ap()
        for k in ("gamma", "beta", "mean", "var")
    }
    pww_d = nc.dram_tensor("pw_w", [COUT, CIN], f32, kind="ExternalInput").ap()
    pwb_d = nc.dram_tensor("pw_b", [COUT, 1], f32, kind="ExternalInput").ap()
    bn2 = {
        k: nc.dram_tensor(f"bn2_{k}", [COUT, 1], f32, kind="ExternalInput").ap()
        for k in ("gamma", "beta", "mean", "var")
    }
    z_d = nc.dram_tensor("z", [BPC, COUT, HW], f32, kind="ExternalOutput").ap()

    with tile.TileContext(nc) as tc:
        with (
            tc.tile_pool(name="const", bufs=1) as const,
            tc.tile_pool(name="stats", bufs=4) as stats,
            tc.tile_pool(name="xp", bufs=3) as xpool,
            tc.tile_pool(name="yp", bufs=3) as ypool,
            tc.tile_pool(name="zp", bufs=4) as zpool,
            tc.tile_pool(name="lmp", bufs=2) as lmpool,
        ):
            # ---------------- setup: params ----------------
            ident = const.tile([128, 128], f32, tag="ident")
            make_identity(nc, ident[:])

            def load(pool, dram, shape, tag):
                t = pool.tile(shape, f32, tag=tag)
                nc.sync.dma_start(t[:], dram)
                return t

            dww = load(const, dww_d[:], [128, 9], "dww")
            dwb = load(const, dwb_d[:], [128, 1], "dwb")
            b1 = {k: load(const, v[:], [128, 1], f"b1{k}") for k, v in bn1.items()}
            s1, bias1 = _affine(
                nc, const, "a1", b1["var"][:], b1["gamma"][:], b1["beta"][:],
                b1["mean"][:], dwb[:],
            )

            # depthwise tap diagonals, BN1 scale folded in
            wsc = const.tile([128, 9], f32, tag="wsc")
            nc.vector.tensor_scalar(wsc[:], dww[:], s1[:], None, Alu.mult)
            dmats = []
            for ti, (dh, dw) in enumerate(TAPS):
                tap_col = (dh + 1) * 3 + (dw + 1)
                d = const.tile([128, 128], f32r, tag=f"d{ti}")
                nc.vector.tensor_scalar(
                    d[:], ident[:], wsc[:, tap_col:tap_col + 1], None, Alu.mult
                )
                dmats.append(d)

            # pointwise weights: scale rows by BN2 s2, then transpose on PE
            pwT = const.tile([128, 256], f32, tag="pwT")
            T2 = []
            with tc.tile_pool(name="pstr", bufs=1, space="PSUM") as pstr:
                for ob in range(2):
                    sl = slice(ob * 128, (ob + 1) * 128)
                    pw = load(const, pww_d[sl, :], [128, 128], f"pw{ob}")
                    pwb = load(const, pwb_d[sl, :], [128, 1], f"pwb{ob}")
                    b2 = {
                        k: load(const, v[sl, :], [128, 1], f"b2{k}{ob}")
                        for k, v in bn2.items()
                    }
                    s2, t2 = _affine(
                        nc, const, f"a2{ob}", b2["var"][:], b2["gamma"][:],
                        b2["beta"][:], b2["mean"][:], pwb[:],
                    )
                    T2.append(t2)
                    nc.vector.tensor_scalar(pw[:], pw[:], s2[:], None, Alu.mult)
                    pt = pstr.tile([128, 128], f32, tag="pt")
                    nc.tensor.transpose(pt[:], pw[:], ident[:])
                    nc.vector.tensor_copy(pwT[:, sl], pt[:])

            # scratch target for the fused max-accum ops (value discarded)
            scr = const.tile([128, 2, TR, 64], f32, tag="scr")

            with (
                tc.tile_pool(name="psdw", bufs=2, space="PSUM") as psdw,
                tc.tile_pool(name="pspw", bufs=2, space="PSUM") as pspw,
            ):
                def dw_stage(b):
                    x_sb = xpool.tile([128, HWP], f32r, tag="x")
                    ck = 14 * WP
                    for c in range(4):
                        nc.sync.dma_start(
                            x_sb[:, c * ck:(c + 1) * ck],
                            x_d[b, :, c * ck:(c + 1) * ck],
                        )
                    x3 = x_sb.rearrange("p (h w) -> p h w", h=H)
                    y_sb = ypool.tile([128, HW], f32r, tag="y")
                    y4 = y_sb.rearrange("p (t r w) -> p t r w", t=NT, r=TR)
                    mp = stats.tile([128, 4], f32, tag="mp1")
                    for k in range(4):
                        n_t = min(2, NT - 2 * k)
                        ps = psdw.tile([128, 2, TR, 64], f32, tag="psdw")
                        for half in range(n_t):
                            t = 2 * k + half
                            r0 = TR * t
                            for ti, (dh, dw) in enumerate(TAPS):
                                a = max(r0, -dh)
                                bb = min(r0 + TR, 56 - max(0, dh))
                                nc.tensor.matmul(
                                    ps[:, half, a - r0:bb - r0, 0:56],
                                    dmats[ti][:],
                                    x3[:, a + dh:bb + dh, 1 + dw:57 + dw],
                                    start=(ti == 0),
                                    stop=(ti == len(TAPS) - 1),
                                )
                        nc.scalar.activation(
                            y4[:, 2 * k:2 * k + n_t],
                            ps[:, 0:n_t, :, 0:56],
                            Act.Relu,
                            bias=bias1[:],
                        )
                        nc.vector.tensor_scalar(
                            scr[:, 0:n_t, :, 0:56],
                            y4[:, 2 * k:2 * k + n_t],
                            0.0, None, Alu.add,
                            op1=Alu.max,
                            accum_out=mp[:, k:k + 1],
                        )
                    ymax = stats.tile([128, 1], f32, tag="ymax")
                    nc.vector.tensor_reduce(ymax[:], mp[:], axis=AxL.X, op=Alu.max)
                    mask1 = stats.tile([128, 1], f32, tag="mask1")
                    nc.vector.tensor_scalar(mask1[:], ymax[:], DW_T, None, Alu.is_ge)
                    lm = lmpool.tile([128, 256], f32r, tag="lm")
                    nc.vector.tensor_scalar(
                        lm[:, 0:128], pwT[:, 0:128], mask1[:], None, Alu.mult
                    )
                    nc.vector.tensor_scalar(
                        lm[:, 128:256], pwT[:, 128:256], mask1[:], None, Alu.mult
                    )
                    return y4, lm

                def pw_stage(b, y4, lm):
                    for ob in range(2):
                        z_sb = zpool.tile([128, HW], f32, tag="z")
                        z4 = z_sb.rearrange("p (t r w) -> p t r w", t=NT, r=TR)
                        mpz = stats.tile([128, 4], f32, tag="mpz")
                        for k in range(4):
                            n_t = min(2, NT - 2 * k)
                            ps = pspw.tile([128, 2, TR, 64], f32, tag="pspw")
                            for half in range(n_t):
                                t = 2 * k + half
                                nc.tensor.matmul(
                                    ps[:, half, :, 0:56],
                                    lm[:, ob * 128:(ob + 1) * 128],
                                    y4[:, t],
                                    start=True,
                                    stop=True,
                                )
                            nc.scalar.activation(
                                z4[:, 2 * k:2 * k + n_t],
                                ps[:, 0:n_t, :, 0:56],
                                Act.Relu,
                                bias=T2[ob][:],
                            )
                            nc.vector.tensor_scalar(
                                scr[:, 0:n_t, :, 0:56],
                                z4[:, 2 * k:2 * k + n_t],
                                0.0, None, Alu.add,
                                op1=Alu.max,
                                accum_out=mpz[:, k:k + 1],
                            )
                        zmax = stats.tile([128, 1], f32, tag="zmax")
                        nc.vector.tensor_reduce(zmax[:], mpz[:], axis=AxL.X, op=Alu.max)
                        maskz = stats.tile([128, 1], f32, tag="maskz")
                        nc.vector.tensor_scalar(maskz[:], zmax[:], PW_T, None, Alu.is_ge)
                        for c in range(2):
                            seg = z_sb[:, c * 1568:(c + 1) * 1568]
                            nc.gpsimd.tensor_scalar(seg, seg, maskz[:], None, Alu.mult)
                            nc.sync.dma_start(
                                z_d[b, ob * 128:(ob + 1) * 128,
                                    c * 1568:(c + 1) * 1568],
                                seg,
                            )

                # software-pipelined sample loop: DW(b+1) traced before PW(b)
                # so the PE never stalls waiting for the prune mask.
                state = {}
                state[0] = dw_stage(0)
                for b in range(BPC):
                    if b + 1 < BPC:
                        state[b + 1] = dw_stage(b + 1)
                    pw_stage(b, *state.pop(b))

    nc.compile()
    return nc


_NC_CACHE = None


def make_in_maps(inputs):
    def f(name, shape):
        return np.ascontiguousarray(
            np.asarray(inputs[name], dtype=np.float32).reshape(shape)
        )

    x = f("x", (B, CIN, H, W))
    xp = np.zeros((B, CIN, H, WP), dtype=np.float32)
    xp[:, :, :, 1:57] = x
    xp = xp.reshape(B, CIN, HWP)
    base = {
        "dw_w": f("dw_w", (CIN, 9)),
        "dw_b": f("dw_b", (CIN, 1)),
        "bn1_gamma": f("bn1_gamma", (CIN, 1)),
        "bn1_beta": f("bn1_beta", (CIN, 1)),
        "bn1_mean": f("bn1_mean", (CIN, 1)),
        "bn1_var": f("bn1_var", (CIN, 1)),
        "pw_w": f("pw_w", (COUT, CIN)),
        "pw_b": f("pw_b", (COUT, 1)),
        "bn2_gamma": f("bn2_gamma", (COUT, 1)),
        "bn2_beta": f("bn2_beta", (COUT, 1)),
        "bn2_mean": f("bn2_mean", (COUT, 1)),
        "bn2_var": f("bn2_var", (COUT, 1)),
    }
    return [
        {"x": np.ascontiguousarray(xp[i * BPC:(i + 1) * BPC]), **base}
        for i in range(N_CORES)
    ]


def kernel(**inputs) -> np.ndarray:
    global _NC_CACHE
    if _NC_CACHE is None:
        _NC_CACHE = build()
    nc = _NC_CACHE
    in_maps = make_in_maps(inputs)
    res = run_bass_kernel_spmd(nc, in_maps, core_ids=list(range(N_CORES)))
    out = np.concatenate([r["z"] for r in res.results], axis=0)
    return out.reshape(B, COUT, H, W)


if __name__ == "__main__":
    build()
    print("build ok")
